# revision 1
# baseline (speedup 1.0000x reference)
"""Correlation layer + softmax(axis=i) Trainium2 kernel.

corr[b,i,j] = sum_c f1[b,c,i] * f2[b,c,j]   (b=4, c=256, i,j in hw=4096)
out = softmax(corr, axis=i) reshaped to (4, 4096, 64, 64)

Sharding: 8 cores = 4 batches x 2 j-halves. Softmax reduces over i, which is
fully local per core when corr is computed transposed (j on partitions, i on
the free axis). Per core, for each of 16 j-tiles (128 j's):
  1. corrT tile (128 j, 4096 i) = f2_tile.T @ f1 via 16 matmuls (fp32r,
     1 cyc/row) into 8 resident PSUM banks,
  2. per-column max via reduce_max straight off PSUM (negated -> exp bias),
  3. exp(corr - max) PSUM -> SBUF on ACT, accumulating row sums on the fly,
  4. 1/sum via an ACT-only chain (recip = exp(-ln(total))), normalize in
     place on the Pool engine, DMA the (128 j, 4096 i) tile out contiguous.
The device output is (2048 j, 4096 i) per core; the host transposes during
unsharding (the gather has to copy these bytes anyway).

This walrus build allows only ONE sync wait per instruction. Tile freely
emits several, so kernel.py patches two spots in the Tile pipeline:
  - a post-scheduling pass splits every multi-wait instruction into
    single-wait same-engine Drain carriers ahead of it,
  - the kernel-tail drain (one wait per outstanding semaphore) is split the
    same way.
"""

import sys

import numpy as np

sys.path.insert(0, "/opt/trn_rl_repo")

import concourse.bass as bass
import concourse.mybir as mybir
import concourse.tile as tile
from concourse.bass_utils import run_bass_kernel_spmd

B, C, H, W = 4, 256, 64, 64
HW = H * W  # 4096
JJ = HW // 2  # j columns per core
N_CORES = 8
P = 128
KC = C // P  # 2 contraction chunks
NJT = JJ // P  # 16 j-tiles per core
IC = 512  # i-chunk width (one PSUM bank)
NIC = HW // IC  # 8 i-chunks
MMN = 512  # matmul moving width (one PSUM bank)
USE_FP32R = True  # fp32r matmul: 1 cyc/row vs 4 for fp32

FP32 = mybir.dt.float32

_split_counter = [0]


def _split_multiwaits(ordered):
    """Walrus (this build) rejects instructions with >1 sync wait. Hoist the
    extra waits onto single-wait Drain instructions on the same engine queue
    immediately before the offender (queues are in-order)."""
    for bb, insts in ordered.items():
        out = []
        changed = False
        for inst in insts:
            si = getattr(inst, "sync_info", None)
            waits = list(si.on_wait) if (si is not None and si.on_wait) else []
            if len(waits) > 1:
                changed = True
                for w in waits[:-1]:
                    _split_counter[0] += 1
                    d = mybir.InstDrain(
                        name=f"I-wsplit-{_split_counter[0]}",
                        ins=[],
                        outs=[],
                        engine=inst.engine,
                    )
                    d.sync_info = mybir.SyncInfo(on_wait=[w], on_update=[])
                    out.append(d)
                si.on_wait = waits[-1:]
            out.append(inst)
        if changed:
            ordered[bb] = out
    return ordered


_orig_postorder = tile.postorder_instruction_blocks


def _patched_postorder(ordered, start_bb_name, postordered):
    _split_multiwaits(ordered)
    return _orig_postorder(ordered, start_bb_name, postordered)


tile.postorder_instruction_blocks = _patched_postorder


def _patched_drain_and_barrier(self, tick_clock, wait_clock):
    """Same single-wait discipline for the kernel-tail drain."""
    from concourse.vector_clock import ScopedClock

    drain_inst = self.nc.sync.drain()
    wait_clock.add_sem_waits(
        drain_inst.ins, ScopedClock({None: tick_clock.global_clock})
    )
    si = drain_inst.ins.sync_info
    waits = list(si.on_wait or []) if si is not None else []
    if len(waits) > 1:
        si.on_wait = waits[:1]
        for w in waits[1:]:
            d2 = self.nc.sync.drain()
            si2 = d2.ins.sync_info
            if si2 is None:
                d2.ins.sync_info = mybir.SyncInfo(on_wait=[w], on_update=[])
            else:
                si2.on_wait = [w]
    self.nc.all_engine_barrier()
    assert self.sems is not None
    popped = self.nc._tile_sem_poison_stack.pop()
    assert popped is self._sem_poison
    self.nc.clear_and_free_semaphores(list(self.sems.allocated().values()))
    self.nc.all_engine_barrier()


tile.TileContext._drain_and_barrier = _patched_drain_and_barrier


def _build_bass():
    nc = bass.Bass()
    mmdt = mybir.dt.float32r if USE_FP32R else FP32
    fin = nc.declare_dram_parameter("fin", [C, HW + JJ], mmdt, isOutput=False)
    out = nc.declare_dram_parameter("out", [JJ, HW], FP32, isOutput=True)

    with tile.TileContext(nc) as tc:
        with (
            tc.tile_pool(name="singles", bufs=1) as singles,
            tc.tile_pool(name="exp", bufs=3) as exp_pool,
            tc.tile_pool(name="stats", bufs=16) as stats,
            tc.tile_pool(name="ps", bufs=8, space="PSUM") as ps_pool,
        ):
            # Preload f1|f2, one SBUF tile per 128-row contraction chunk.
            # SWDGE (gpsimd) keeps the SP queue free for the output stream.
            fin_sb = []
            for cc in range(KC):
                t = singles.tile([P, HW + JJ], mmdt, tag=f"fin_{cc}")
                eng = nc.gpsimd if cc == 0 else nc.scalar
                eng.dma_start(out=t, in_=fin[cc * P : (cc + 1) * P, :])
                fin_sb.append(t)

            for jt in range(NJT):
                # 1. corrT j-tile into 8 resident PSUM banks
                ps_list = []
                for ic in range(NIC):
                    ps = ps_pool.tile([P, IC], FP32, tag="ps")
                    for sub in range(IC // MMN):
                        for cc in range(KC):
                            nc.tensor.matmul(
                                ps[:, bass.ts(sub, MMN)],
                                lhsT=fin_sb[cc][:, HW + jt * P : HW + (jt + 1) * P],
                                rhs=fin_sb[cc][
                                    :, ic * IC + sub * MMN : ic * IC + (sub + 1) * MMN
                                ],
                                start=(cc == 0),
                                stop=(cc == KC - 1),
                            )
                    ps_list.append(ps)
                # 2. per-column (per-partition here) max, straight off PSUM
                mx = stats.tile([P, NIC], FP32)
                for ic in range(NIC):
                    nc.vector.reduce_max(
                        out=mx[:, ic : ic + 1],
                        in_=ps_list[ic],
                        axis=mybir.AxisListType.X,
                    )
                negmax = stats.tile([P, 1], FP32)
                nc.vector.reduce_max(
                    out=negmax, in_=mx, axis=mybir.AxisListType.X, negate=True
                )
                # 3. exp(corr - max) PSUM -> SBUF, accumulating row sums
                exp_t = exp_pool.tile([P, HW], FP32)
                sums = stats.tile([P, NIC], FP32)
                for ic in range(NIC):
                    nc.scalar.activation(
                        out=exp_t[:, bass.ts(ic, IC)],
                        in_=ps_list[ic],
                        func=mybir.ActivationFunctionType.Exp,
                        bias=negmax,
                        scale=1.0,
                        accum_out=sums[:, ic : ic + 1],
                    )
                # 4. recip = exp(-ln(total)) via ACT-only chain, normalize in
                # place on Pool, stream out over the HWDGE queues.
                total = stats.tile([P, 1], FP32)
                nc.vector.reduce_sum(out=total, in_=sums, axis=mybir.AxisListType.X)
                lntot = stats.tile([P, 1], FP32)
                nc.scalar.activation(
                    out=lntot, in_=total, func=mybir.ActivationFunctionType.Ln
                )
                recip = stats.tile([P, 1], FP32)
                nc.scalar.activation(
                    out=recip,
                    in_=lntot,
                    func=mybir.ActivationFunctionType.Exp,
                    scale=-1.0,
                )
                nc.gpsimd.tensor_scalar_mul(out=exp_t, in0=exp_t, scalar1=recip)
                # 5. fully contiguous DMA out (row j = jt*128 + p)
                nc.sync.dma_start(
                    out=out[jt * P : (jt + 1) * P, :],
                    in_=exp_t,
                )
    return nc


_NC = None


def _get_nc():
    global _NC
    if _NC is None:
        _NC = _build_bass()
    return _NC


def _run(feat1, feat2, trace=False):
    f1 = np.asarray(feat1, dtype=np.float32).reshape(B, C, HW)
    f2 = np.asarray(feat2, dtype=np.float32).reshape(B, C, HW)
    in_maps = []
    for d in range(N_CORES):
        bb, jh = d // 2, d % 2
        fin = np.concatenate([f1[bb], f2[bb][:, jh * JJ : (jh + 1) * JJ]], axis=1)
        in_maps.append({"fin": np.ascontiguousarray(fin)})
    res = run_bass_kernel_spmd(_get_nc(), in_maps, list(range(N_CORES)), trace=trace)
    out = np.empty((B, HW, HW), np.float32)
    for d in range(N_CORES):
        bb, jh = d // 2, d % 2
        # device tile is (j_local, i); transpose during unshard
        out[bb][:, jh * JJ : (jh + 1) * JJ] = res.results[d]["out"].T
    return out.reshape(B, HW, H, W), res


def kernel(feat1, feat2):
    out, _ = _run(feat1, feat2)
    return out

